# revision 1
# baseline (speedup 1.0000x reference)
"""Trainium2 Bass kernel: DiT block with cross-attention (nn_DiTBlock_CrossAttn).

Sharding: pure data-parallel over batch. B=8 batch elements -> 8 NeuronCores,
one batch element per core, no collectives. Each core runs the full block:
adaLN -> self-attn -> cross-attn -> FFN (exact GELU).

Layout: activations kept feature-major ("transposed", [feature_part, chunk, token])
so every projection is matmul(lhsT=W[din,dout], rhs=actT[din,n]) with weights in
their natural DRAM layout. Attention uses the S^T orientation with a fused
ones-column in V for the softmax denominator (softmax without max subtraction is
safe: |logits| < ~5 for this problem family). Matmuls run in bf16 (weights are
pre-cast on host), accumulation and residual stream stay fp32.
"""
import contextlib

import numpy as np
import ml_dtypes

import concourse.bass as bass
import concourse.tile as tile
import concourse.mybir as mybir
from concourse import bacc
from concourse.bass_utils import run_bass_kernel_spmd
from concourse.masks import make_identity

P = 128
N = 1024            # tokens
D = 1024            # hidden
KD = D // P         # 8 feature chunks of hidden
NT = N // P         # 8 token tiles
H = 16              # heads
HD = 64             # head dim
S = 256             # context tokens
ST = S // P         # 2
CD = 512            # context dim
CKD = CD // P       # 4
MLP = 4096
MT = MLP // P       # 32
EPS = 1e-6
ASCALE = 0.125      # 1/sqrt(HD)
NCORES = 8

F32 = mybir.dt.float32
BF16 = mybir.dt.bfloat16
AF = mybir.ActivationFunctionType
OP = mybir.AluOpType


def _wcols(w):
    """[din, dout] dram AP -> [p, ko, dout] (feature-chunked lhsT view)."""
    return w.rearrange("(ko p) f -> p ko f", p=P)


def build_nc(taps=(), upto='full'):
    nc = bacc.Bacc("TRN2", target_bir_lowering=False, debug=False)

    d = {}
    d['x'] = nc.dram_tensor("x", [N, D], F32, kind="ExternalInput").ap()
    d['c'] = nc.dram_tensor("c", [D], F32, kind="ExternalInput").ap()
    d['context'] = nc.dram_tensor("context", [S, CD], F32, kind="ExternalInput").ap()
    for nm, sh in [("w_qkv", [D, 3 * D]), ("w_so", [D, D]), ("w_cq", [D, D]),
                   ("w_ck", [CD, D]), ("w_cv", [CD, D]), ("w_co", [D, D]),
                   ("w1", [D, MLP]), ("w2", [MLP, D]), ("w_ada", [D, 6 * D])]:
        d[nm] = nc.dram_tensor(nm, sh, BF16, kind="ExternalInput").ap()
    for nm, sh in [("b_qkv", [3 * D]), ("b_so", [D]), ("b_cq", [D]), ("b_ck", [D]),
                   ("b_cv", [D]), ("b_co", [D]), ("b1", [MLP]), ("b2", [D]),
                   ("b_ada", [6 * D])]:
        d[nm] = nc.dram_tensor(nm, sh, F32, kind="ExternalInput").ap()
    out = nc.dram_tensor("out_x", [N, D], F32, kind="ExternalOutput").ap()
    srows = nc.dram_tensor("srows", [40, N], F32).ap()
    g_dram = nc.dram_tensor("g_dram", [MT, P, N], BF16).ap()

    tap_shapes = {
        "ada": ([P, 48], F32), "h1": ([P, KD, N], BF16),
        "q": ([P, KD, N], BF16), "k": ([P, KD, N], BF16),
        "v65": ([P, NT, H, 65], BF16), "saO": ([P, KD, N], BF16),
        "x2": ([P, KD, N], F32), "h2": ([P, KD, N], BF16),
        "cq": ([P, KD, N], BF16), "ck": ([P, KD, S], BF16),
        "cv65": ([P, ST, H, 65], BF16), "caO": ([P, KD, N], BF16),
        "x3": ([P, KD, N], F32), "h3": ([P, KD, N], BF16),
        "xT": ([P, KD, N], F32),
    }
    tap_aps = {nm: nc.dram_tensor(f"dbg_{nm}", *tap_shapes[nm], kind="ExternalOutput").ap()
               for nm in taps}

    with tile.TileContext(nc) as tc:
        _emit(nc, tc, d, out, srows, g_dram, tap_aps, upto)
    nc.compile()
    return nc


def _emit(nc, tc, d, out, srows, g_dram, tap_aps={}, upto='full'):
    def tap(nm, t):
        if nm in tap_aps:
            nc.sync.dma_start(tap_aps[nm], t[:])

    gl = contextlib.ExitStack()          # global pools, whole-kernel lifetime
    with gl:
        const = gl.enter_context(tc.tile_pool(name="const", bufs=1))
        resid = gl.enter_context(tc.tile_pool(name="resid", bufs=2))
        wpool = gl.enter_context(tc.tile_pool(name="wpool", bufs=3))
        bigbf = gl.enter_context(tc.tile_pool(name="bigbf", bufs=3))

        # ---------- constants ----------
        ident = const.tile([P, P], F32, tag="ident")
        make_identity(nc, ident)
        onesD_mat = const.tile([P, P], BF16, tag="onesD_mat")
        nc.vector.memset(onesD_mat[:], 1.0 / D)
        onesD_row = const.tile([P, 1], BF16, tag="onesD_row")
        nc.vector.memset(onesD_row[:], 1.0 / D)
        ones1_f = const.tile([1, P], F32, tag="ones1_f")
        nc.vector.memset(ones1_f[:], 1.0)
        eps_t = const.tile([P, 1], F32, tag="eps")
        nc.vector.memset(eps_t[:], EPS)

        ctxT = const.tile([P, CKD, S], BF16, tag="ctxT")
        ada = const.tile([P, 48], F32, tag="ada")
        splus = const.tile([P, 24], F32, tag="splus")
        xT = resid.tile([P, KD, N], F32, tag="resid")

        def partial_out(ref_tile):
            for k in range(KD):
                nc.sync.dma_start(out[k * P:(k + 1) * P, :], ref_tile[:, k, :])

        # ---------- staging scope ----------
        st = contextlib.ExitStack()
        stg = st.enter_context(tc.tile_pool(name="stg", bufs=4))
        ps_t = st.enter_context(tc.tile_pool(name="ps_t", bufs=3, space="PSUM"))

        def bias_T(name, brow, width):
            stage = stg.tile([width, P], F32, tag="btmp")
            nc.sync.dma_start(stage[:], brow.rearrange("(a p) -> a p", p=P))
            ps = ps_t.tile([P, 512], F32, tag="pst")
            nc.tensor.transpose(ps[:, 0:width], stage[:], ident[0:width, 0:width])
            t = const.tile([P, width], F32, tag=f"bT_{name}")
            nc.vector.tensor_copy(t[:], ps[:, 0:width])
            return t

        b_qkvT = bias_T("qkv", d['b_qkv'], 24)
        b_soT = bias_T("so", d['b_so'], KD)
        b_cqT = bias_T("cq", d['b_cq'], KD)
        b_ckT = bias_T("ck", d['b_ck'], KD)
        b_coT = bias_T("co", d['b_co'], KD)
        b1T = bias_T("b1", d['b1'], MT)
        b_adaT = bias_T("ada", d['b_ada'], 48)
        b2T = bias_T("b2", d['b2'], KD)

        # x -> xT (feature-major, fp32, via PE transpose)
        for i in range(NT):
            xs = stg.tile([P, D], F32, tag="xstage")
            nc.sync.dma_start(xs[:], d['x'][i * P:(i + 1) * P, :])
            for jg in range(2):
                ps = ps_t.tile([P, 512], F32, tag="pst")
                for j4 in range(4):
                    j = jg * 4 + j4
                    nc.tensor.transpose(ps[:, j4 * 128:(j4 + 1) * 128],
                                        xs[:, j * 128:(j + 1) * 128], ident[:])
                nc.vector.tensor_copy(
                    xT[:, jg * 4:(jg + 1) * 4, i * P:(i + 1) * P],
                    ps.rearrange("p (a b) -> p a b", a=4))

        # context -> ctxT (bf16)
        for i in range(ST):
            cs = stg.tile([P, D], F32, tag="xstage")
            nc.sync.dma_start(cs[:, 0:CD], d['context'][i * P:(i + 1) * P, :])
            ps = ps_t.tile([P, 512], F32, tag="pst")
            for j in range(4):
                nc.tensor.transpose(ps[:, j * 128:(j + 1) * 128],
                                    cs[:, j * 128:(j + 1) * 128], ident[:])
            nc.vector.tensor_copy(
                ctxT[:, :, i * P:(i + 1) * P],
                ps.rearrange("p (a b) -> p a b", a=4))

        # c -> silu(c)^T (bf16, feature-major [P, KD])
        cst = stg.tile([KD, P], F32, tag="cstage")
        nc.sync.dma_start(cst[:], d['c'].rearrange("(a p) -> a p", p=P))
        csil = stg.tile([KD, P], F32, tag="cstage")
        nc.scalar.activation(csil[:], cst[:], AF.Silu)
        pcs = ps_t.tile([P, 512], F32, tag="pst")
        nc.tensor.transpose(pcs[:, 0:KD], csil[:], ident[0:KD, 0:KD])
        silu_cT = const.tile([P, KD], BF16, tag="silu_cT")
        nc.vector.tensor_copy(silu_cT[:], pcs[:, 0:KD])

        # ada = silu(c) @ w_ada + b_ada  -> feature-major [P, 48]
        wada = _wcols(d['w_ada'])
        ada_blocks = 0 if upto == 'stage_noada' else 6
        if upto == 'stage_dmaonly':
            # DMA w_ada blocks but skip the matmuls; consume via tiny copy
            for blk in range(6):
                wb = wpool.tile([P, KD, 1024], BF16, tag="wblk")
                nc.sync.dma_start(wb[:], wada[:, :, blk * 1024:(blk + 1) * 1024])
                nc.vector.tensor_copy(ada[:, blk:blk+1].bitcast(BF16)[:, 0:1], wb[:, 0, 0:1])
            ada_blocks = 0
        if ada_blocks == 0:
            nc.vector.memset(ada[:], 0.01)
        for blk in range(ada_blocks):
            wb = wpool.tile([P, KD, 1024], BF16, tag="wblk")
            nc.sync.dma_start(wb[:], wada[:, :, blk * 1024:(blk + 1) * 1024])
            for t8 in range(8):
                t = blk * 8 + t8
                ps = ps_t.tile([P, 512], F32, tag="pst")
                for k in range(KD):
                    nc.tensor.matmul(ps[:, 0:1], wb[:, k, t8 * 128:(t8 + 1) * 128],
                                     silu_cT[:, k:k + 1],
                                     start=(k == 0), stop=(k == KD - 1))
                nc.vector.tensor_copy(ada[:, t:t + 1], ps[:, 0:1])
        nc.vector.tensor_add(ada[:], ada[:], b_adaT[:])
        for g in range(3):
            nc.vector.tensor_scalar_add(splus[:, g * 8:(g + 1) * 8],
                                        ada[:, g * 16 + 8:g * 16 + 16], 1.0)
        tap("ada", ada)
        tap("xT", xT)
        st.close()

        if upto in ('stage', 'stage_noada', 'stage_dmaonly'):
            partial_out(xT)
            return
        # ---------- LN + modulate (self-contained pool scope) ----------
        def ln_mod(x_in, g):
            h_out = bigbf.tile([P, KD, N], BF16, tag="big")
            ls = contextlib.ExitStack()
            with ls:
                lnb = ls.enter_context(tc.tile_pool(name="lnb", bufs=2))
                lrows = ls.enter_context(tc.tile_pool(name="lrows", bufs=3))
                ps_mu = ls.enter_context(tc.tile_pool(name="ps_mu", bufs=1, space="PSUM"))
                ps_rs = ls.enter_context(tc.tile_pool(name="ps_rs", bufs=1, space="PSUM"))
                ps_e2 = ls.enter_context(tc.tile_pool(name="ps_e2", bufs=1, space="PSUM"))
                # mean lands ALREADY BROADCAST across partitions: lhsT is the
                # all-ones(1/D) matrix, so every psum partition gets mean(x).
                mu_ps = ps_mu.tile([P, N], F32, tag="mups")
                e2_ps = ps_e2.tile([1, N], F32, tag="e2ps")
                for k in range(KD):
                    xbf = lnb.tile([P, N], BF16, tag="lnxbf")
                    nc.vector.tensor_copy(xbf[:], x_in[:, k])
                    sq = lnb.tile([P, N], BF16, tag="lnsq")
                    nc.scalar.activation(sq[:], xbf[:], AF.Square)
                    for half in range(2):
                        hs = slice(half * 512, (half + 1) * 512)
                        nc.tensor.matmul(mu_ps[:, hs], onesD_mat[:], xbf[:, hs],
                                         start=(k == 0), stop=(k == KD - 1))
                        nc.tensor.matmul(e2_ps[:, hs], onesD_row[:], sq[:, hs],
                                         start=(k == 0), stop=(k == KD - 1))
                murow = lrows.tile([1, N], F32, tag="row")
                nc.vector.tensor_copy(murow[:], mu_ps[0:1, :])
                var = lrows.tile([1, N], F32, tag="row")
                nc.vector.tensor_mul(var[:], murow[:], murow[:])
                e2row = lrows.tile([1, N], F32, tag="row")
                nc.vector.tensor_copy(e2row[:], e2_ps[:])
                nc.vector.tensor_sub(var[:], e2row[:], var[:])
                nc.scalar.activation(var[:], var[:], AF.Sqrt, bias=eps_t[0:1])
                nc.vector.reciprocal(var[:], var[:])
                # broadcast rstd across partitions with a K=1 ones matmul
                rs_ps = ps_rs.tile([P, N], F32, tag="rsps")
                for half in range(2):
                    hs = slice(half * 512, (half + 1) * 512)
                    nc.tensor.matmul(rs_ps[:, hs], ones1_f[:], var[:, hs],
                                     start=True, stop=True)
                for k in range(KD):
                    t1 = lnb.tile([P, N], F32, tag="lnt1")
                    nc.vector.tensor_sub(t1[:], x_in[:, k], mu_ps[:])
                    nc.vector.tensor_mul(t1[:], t1[:], rs_ps[:])
                    nc.gpsimd.tensor_scalar(h_out[:, k], t1[:],
                                            splus[:, g * 8 + k:g * 8 + k + 1],
                                            ada[:, g * 16 + k:g * 16 + k + 1],
                                            OP.mult, OP.add)
            return h_out

        # ---------- generic transposed projection (512-wide weight blocks) ----
        def proj_T(ps_mm, w_cols, kdin, act_bf, n_free, dout, evict):
            nhalf = max(1, n_free // 512)
            for blk in range(dout // 1024):
                wb = wpool.tile([P, kdin, 1024], BF16, tag="wblk")
                nc.sync.dma_start(wb[:], w_cols[:, :, blk * 1024:(blk + 1) * 1024])
                for t8 in range(8):
                    ps = ps_mm.tile([P, N], F32, tag="pmm")
                    for half in range(nhalf):
                        hs = slice(half * 512, half * 512 + min(512, n_free))
                        for k in range(kdin):
                            nc.tensor.matmul(ps[:, hs],
                                             wb[:, k, t8 * 128:(t8 + 1) * 128],
                                             act_bf[:, k, hs],
                                             start=(k == 0), stop=(k == kdin - 1))
                    evict(blk * 8 + t8, ps)

        # ---------- token-major V projection (fused ones column) ----------
        def proj_V(ps_mm, w_cols, kdin, act_bf, m_tiles, v65, bias_b):
            wb = wpool.tile([P, kdin, 1024], BF16, tag="wblk")
            nc.sync.dma_start(wb[:], w_cols[:])
            for blk in range(2):
                for i in range(m_tiles):
                    ps = ps_mm.tile([P, N], F32, tag="pmm")
                    for k in range(kdin):
                        nc.tensor.matmul(ps[:, 0:512],
                                         act_bf[:, k, i * 128:(i + 1) * 128],
                                         wb[:, k, blk * 512:(blk + 1) * 512],
                                         start=(k == 0), stop=(k == kdin - 1))
                    nc.vector.tensor_add(
                        v65[:, i, blk * 8:(blk + 1) * 8, 0:64],
                        ps[:, 0:512].rearrange("p (h e) -> p h e", h=8),
                        bias_b[:, blk * 512:(blk + 1) * 512]
                        .rearrange("p (h e) -> p h e", h=8))
            nc.vector.memset(v65[:, :, :, 64:65], 1.0)

        # ---------- attention core (self-contained pool scope) ----------
        def attention(q_T, kv_T, v65, m_tiles, o_bf, srow_base):
            at = contextlib.ExitStack()
            with at:
                expp = at.enter_context(tc.tile_pool(name="expp", bufs=6))
                arows = at.enter_context(tc.tile_pool(name="arows", bufs=2))
                rb = at.enter_context(tc.tile_pool(name="rb", bufs=2))
                ps_lg = at.enter_context(tc.tile_pool(name="ps_lg", bufs=2, space="PSUM"))
                ps_pv = at.enter_context(tc.tile_pool(name="ps_pv", bufs=3, space="PSUM"))
                for h in range(H):
                    pr, off = h // 2, (h % 2) * 64
                    pv = ps_pv.tile([65, N], F32, tag="pv")
                    # software pipeline over (mt, half) items: logits+exp run two
                    # items ahead of the PV accumulation so the PE never waits on
                    # the ACT exp eviction.
                    items = m_tiles * 2
                    exs = [None] * items
                    def lgexp(i):
                        mt, half = i // 2, i % 2
                        hs = slice(half * 512, (half + 1) * 512)
                        lg = ps_lg.tile([P, 512], F32, tag="lg", name=f"lg{h}_{i}")
                        nc.tensor.matmul(
                            lg[:],
                            kv_T[off:off + 64, pr, mt * 128:(mt + 1) * 128],
                            q_T[off:off + 64, pr, hs],
                            start=True, stop=True)
                        ex = expp.tile([P, 512], BF16, tag="expT", name=f"ex{h}_{i}")
                        nc.scalar.activation(ex[:], lg[:], AF.Exp, scale=ASCALE)
                        exs[i] = ex
                    def pvacc(i):
                        mt, half = i // 2, i % 2
                        hs = slice(half * 512, (half + 1) * 512)
                        nc.tensor.matmul(pv[:, hs], v65[:, mt, h, :], exs[i][:],
                                         start=(mt == 0), stop=(mt == m_tiles - 1))
                    for i in range(items + 2):
                        if i < items:
                            lgexp(i)
                        if i >= 2:
                            pvacc(i - 2)
                    rec = arows.tile([1, N], F32, tag="row")
                    nc.vector.reciprocal(rec[:], pv[64:65, :])
                    r = srow_base + h
                    nc.sync.dma_start(srows[r:r + 1, :], rec[:])
                    rbt = rb.tile([64, N], F32, tag="rbt")
                    nc.sync.dma_start(rbt[:], srows[r:r + 1, :].partition_broadcast(64))
                    nc.vector.tensor_mul(o_bf[off:off + 64, pr, :], pv[0:64, :], rbt[:])

        # ================= self-attention =================
        h1 = ln_mod(xT, 0)

        qT = bigbf.tile([P, KD, N], BF16, tag="big")
        kT = bigbf.tile([P, KD, N], BF16, tag="big")

        sa_es = contextlib.ExitStack()
        vp = sa_es.enter_context(tc.tile_pool(name="vp", bufs=1))
        vb = sa_es.enter_context(tc.tile_pool(name="vb", bufs=1))
        v65 = vp.tile([P, NT, H, 65], BF16, tag="v65")
        vbias_b = vb.tile([P, D], F32, tag="vbias")
        nc.sync.dma_start(vbias_b[:],
                            d['b_qkv'][2 * D:3 * D][None, :].partition_broadcast(P))

        qkv_ps = contextlib.ExitStack()
        ps_mm = qkv_ps.enter_context(tc.tile_pool(name="ps_mm", bufs=2, space="PSUM"))

        def ev_qk(t, ps):
            dst = qT if t < 8 else kT
            nc.scalar.activation(dst[:, t % 8, :], ps[:], AF.Identity,
                                 bias=b_qkvT[:, t:t + 1])
        proj_V(ps_mm, _wcols(d['w_qkv'])[:, :, 2 * D:3 * D], KD, h1, NT, v65, vbias_b)
        proj_T(ps_mm, _wcols(d['w_qkv'])[:, :, 0:2 * D], KD, h1, N, 2 * D, ev_qk)
        tap("h1", h1); tap("q", qT); tap("k", kT); tap("v65", v65)
        qkv_ps.close()

        if upto == 'qkv':
            sa_es.close()
            partial_out(xT)
            return
        saO = bigbf.tile([P, KD, N], BF16, tag="big")
        attention(qT, kT, v65, NT, saO, 6)
        tap("saO", saO)
        sa_es.close()
        if upto == 'sa':
            partial_out(xT)
            return

        x2T = resid.tile([P, KD, N], F32, tag="resid")
        so_ps = contextlib.ExitStack()
        ps_mm = so_ps.enter_context(tc.tile_pool(name="ps_mm", bufs=2, space="PSUM"))

        def ev_so(t, ps):
            nc.vector.tensor_scalar_add(x2T[:, t, :], ps[:], b_soT[:, t:t + 1])
            nc.vector.tensor_add(x2T[:, t, :], x2T[:, t, :], xT[:, t, :])
        proj_T(ps_mm, _wcols(d['w_so']), KD, saO, N, D, ev_so)
        tap("x2", x2T)
        so_ps.close()

        # ================= cross-attention =================
        # ck/cv depend only on ctxT -> emit BEFORE LN2 so the PE has work
        # during the LN serial chain (own small psum pool; 2 banks).
        ca_es = contextlib.ExitStack()
        kp = ca_es.enter_context(tc.tile_pool(name="kp", bufs=1))
        vp = ca_es.enter_context(tc.tile_pool(name="vp2", bufs=1))
        vb = ca_es.enter_context(tc.tile_pool(name="vb2", bufs=1))
        ckT = kp.tile([P, KD, S], BF16, tag="ckT")
        cv65 = vp.tile([P, ST, H, 65], BF16, tag="cv65")
        cvbias_b = vb.tile([P, D], F32, tag="cvbias")
        nc.sync.dma_start(cvbias_b[:], d['b_cv'][None, :].partition_broadcast(P))

        ckcv_ps = contextlib.ExitStack()
        ps_kv = ckcv_ps.enter_context(tc.tile_pool(name="ps_kv", bufs=1, space="PSUM"))

        def ev_ck(t, ps):
            nc.scalar.activation(ckT[:, t, :], ps[:, 0:S], AF.Identity,
                                 bias=b_ckT[:, t:t + 1])
        proj_T(ps_kv, _wcols(d['w_ck']), CKD, ctxT, S, D, ev_ck)
        proj_V(ps_kv, _wcols(d['w_cv']), CKD, ctxT, ST, cv65, cvbias_b)
        tap("ck", ckT); tap("cv65", cv65)

        h2 = ln_mod(x2T, 1)
        ckcv_ps.close()

        cqT = bigbf.tile([P, KD, N], BF16, tag="big")
        ca_ps = contextlib.ExitStack()
        ps_mm = ca_ps.enter_context(tc.tile_pool(name="ps_mm", bufs=2, space="PSUM"))

        def ev_cq(t, ps):
            nc.scalar.activation(cqT[:, t, :], ps[:], AF.Identity,
                                 bias=b_cqT[:, t:t + 1])
        proj_T(ps_mm, _wcols(d['w_cq']), KD, h2, N, D, ev_cq)
        tap("h2", h2); tap("cq", cqT)
        ca_ps.close()

        caO = bigbf.tile([P, KD, N], BF16, tag="big")
        attention(cqT, ckT, cv65, ST, caO, 22)
        tap("caO", caO)
        ca_es.close()

        x3T = resid.tile([P, KD, N], F32, tag="resid")
        co_ps = contextlib.ExitStack()
        ps_mm = co_ps.enter_context(tc.tile_pool(name="ps_mm", bufs=2, space="PSUM"))

        def ev_co(t, ps):
            nc.vector.tensor_scalar_add(x3T[:, t, :], ps[:], b_coT[:, t:t + 1])
            nc.vector.tensor_add(x3T[:, t, :], x3T[:, t, :], x2T[:, t, :])
        proj_T(ps_mm, _wcols(d['w_co']), KD, caO, N, D, ev_co)
        tap("x3", x3T)
        co_ps.close()

        if upto == 'ca':
            partial_out(x3T)
            return
        # ================= FFN =================
        h3 = ln_mod(x3T, 2)
        # fold b2 into the residual before the final transpose-accumulate
        for k in range(KD):
            nc.vector.tensor_scalar_add(x3T[:, k, :], x3T[:, k, :], b2T[:, k:k + 1])

        w1_es = contextlib.ExitStack()
        gstage = w1_es.enter_context(tc.tile_pool(name="gstage", bufs=3))
        ps_mm = w1_es.enter_context(tc.tile_pool(name="ps_mm", bufs=2, space="PSUM"))

        def ev_g(t, ps):
            gst = gstage.tile([P, N], BF16, tag="gst")
            nc.scalar.activation(gst[:], ps[:], AF.Gelu, bias=b1T[:, t:t + 1])
            nc.sync.dma_start(g_dram[t], gst[:])
        proj_T(ps_mm, _wcols(d['w1']), KD, h3, N, MLP, ev_g)
        tap("h3", h3)
        w1_es.close()

        if upto == 'w1':
            partial_out(x3T)
            return
        w2_es = contextlib.ExitStack()
        ghp = w2_es.enter_context(tc.tile_pool(name="ghp", bufs=1))
        outst = w2_es.enter_context(tc.tile_pool(name="outst", bufs=3))
        ps_tt = w2_es.enter_context(tc.tile_pool(name="ps_tt", bufs=2, space="PSUM"))
        ps_po = w2_es.enter_context(tc.tile_pool(name="ps_po", bufs=4, space="PSUM"))

        # token-major copy of x3 (+b2), built via PE transposes into bigbf slots
        xtok = [bigbf.tile([P, 4, D], F32, tag="big", name=f"xtok{_i}")
                for _i in range(2)]
        for i in range(NT):
            dst = xtok[i // 4]
            for jg in range(2):
                ps = ps_tt.tile([P, 512], F32, tag="ptt")
                for j4 in range(4):
                    j = jg * 4 + j4
                    nc.tensor.transpose(ps[:, j4 * 128:(j4 + 1) * 128],
                                        x3T[:, j, i * P:(i + 1) * P], ident[:])
                nc.vector.tensor_copy(dst[:, i % 4, jg * 512:(jg + 1) * 512], ps[:])

        w2cols = d['w2'].rearrange("(ko p) f -> p ko f", p=P)
        for nh in range(2):
            gh = ghp.tile([P, MT, 512], BF16, tag="gh")
            for k in range(MT):
                nc.sync.dma_start(gh[:, k, :], g_dram[k, :, nh * 512:(nh + 1) * 512])
            for dq in range(4):
                w2q = wpool.tile([P, MT, 256], BF16, tag="wblk")
                nc.sync.dma_start(w2q[:], w2cols[:, :, dq * 256:(dq + 1) * 256])
                for i4 in range(4):
                    i = nh * 4 + i4
                    po = ps_po.tile([P, 256], F32, tag="po")
                    for k in range(MT):
                        nc.tensor.matmul(po[:], gh[:, k, i4 * 128:(i4 + 1) * 128],
                                         w2q[:, k, :],
                                         start=(k == 0), stop=(k == MT - 1))
                    ost = outst.tile([P, 256], F32, tag="ost")
                    nc.vector.tensor_add(
                        ost[:], po[:],
                        xtok[i // 4][:, i % 4, dq * 256:(dq + 1) * 256])
                    nc.sync.dma_start(out[i * P:(i + 1) * P, dq * 256:(dq + 1) * 256],
                                      ost[:])
        w2_es.close()


_NC = None


def _get_nc():
    global _NC
    if _NC is None:
        _NC = build_nc()
    return _NC


def make_in_maps(inputs):
    wnames = ["w_qkv", "w_so", "w_cq", "w_ck", "w_cv", "w_co", "w1", "w2", "w_ada"]
    bnames = ["b_qkv", "b_so", "b_cq", "b_ck", "b_cv", "b_co", "b1", "b2", "b_ada"]
    shared = {}
    for nm in wnames:
        shared[nm] = np.ascontiguousarray(
            np.asarray(inputs[nm]).astype(ml_dtypes.bfloat16))
    for nm in bnames:
        shared[nm] = np.ascontiguousarray(np.asarray(inputs[nm], dtype=np.float32))
    x = np.asarray(inputs['x'], dtype=np.float32)
    c = np.asarray(inputs['c'], dtype=np.float32)
    ctxt = np.asarray(inputs['context'], dtype=np.float32)
    in_maps = []
    for i in range(NCORES):
        m = dict(shared)
        m['x'] = np.ascontiguousarray(x[i])
        m['c'] = np.ascontiguousarray(c[i])
        m['context'] = np.ascontiguousarray(ctxt[i])
        in_maps.append(m)
    return in_maps


def kernel(**inputs):
    nc = _get_nc()
    in_maps = make_in_maps(inputs)
    res = run_bass_kernel_spmd(nc, in_maps, core_ids=list(range(NCORES)))
    return np.stack([res.results[i]["out_x"] for i in range(NCORES)]).astype(np.float32)


if __name__ == "__main__":
    data = np.load("/root/problem/inputs.npz")
    out = kernel(**{k: data[k] for k in data.files})
    gold = np.load("/root/problem/gold64.npy")
    err = np.abs(out - gold)
    print("max abs err:", err.max(), " rel:", err.max() / np.abs(gold).max())



# revision 19
# speedup vs baseline: 1.6205x; 1.6205x over previous
"""Trainium2 Bass kernel: DiT block with cross-attention (nn_DiTBlock_CrossAttn).

Sharding: pure data-parallel over batch. B=8 batch elements -> 8 NeuronCores,
one batch element per core, no collectives. Each core runs the full block:
adaLN -> self-attn -> cross-attn -> FFN (exact GELU).

Precision plan (validated against the fp64 reference in emulation):
- Attention-side GEMMs (qkv, so, cq, ck, cv, co, ada, PV) run fp8e4m3 with
  MatmulPerfMode.DoubleRow (two 128-deep k-subtiles per instruction, 2x PE
  throughput). Weights are pre-scaled x64 on host; activations are scaled x8
  (x4 for exp) on device; every eviction folds the inverse scale in.
- FFN (w1/w2) stays bf16: quantizing it pushes max-rel-err past the 2e-2 gate.
- Residual stream and LN statistics in bf16 (fp32 psum accumulation).
- QK^T logits stay bf16 (head_dim=64 contraction cannot use DoubleRow).

Layout: activations feature-major ([feature_part, chunk, token]) so every
projection is matmul(lhsT=W[din,dout], rhs=actT[din,n]) with weights in their
natural DRAM layout. Softmax uses a fused ones-column in V for the denominator
(no max subtraction: |logits|<~3.5 for this problem family); the reciprocal row
is broadcast across partitions with a K=1 PE matmul (no DRAM roundtrip).
FFN hidden activations stay SBUF-resident (no g roundtrip through DRAM).
"""
import contextlib

import numpy as np
import ml_dtypes

import concourse.bass as bass
import concourse.tile as tile
import concourse.mybir as mybir
from concourse import bacc
from concourse.bass_utils import run_bass_kernel_spmd
from concourse.masks import make_identity

P = 128
N = 1024            # tokens
D = 1024            # hidden
KD = D // P         # 8 feature chunks of hidden
NT = N // P         # 8 token tiles
H = 16              # heads
HD = 64             # head dim
S = 256             # context tokens
ST = S // P         # 2
CD = 512            # context dim
CKD = CD // P       # 4
MLP = 4096
MT = MLP // P       # 32
EPS = 1e-6
ASCALE = 0.125      # 1/sqrt(HD)
NCORES = 8

# fp8 scales. Weights are scaled x SW on host; activations x SA on device.
SW = 64.0           # attn-side weights
SA = 8.0            # h1/h2/o8/context/silu_c activations
SV = 8.0            # v / cv
SP = 4.0            # exp(logits)
LN4 = float(np.log(SP))
EV = 1.0 / (SW * SA)   # psum descale for fp8 projections (1/512)

F32 = mybir.dt.float32
BF16 = mybir.dt.bfloat16
F8 = mybir.dt.float8e4
AF = mybir.ActivationFunctionType
OP = mybir.AluOpType
DR = mybir.MatmulPerfMode.DoubleRow

F8_WNAMES = ["w_qkv", "w_so", "w_cq", "w_ck", "w_cv", "w_co", "w_ada"]
BF_WNAMES = ["w1", "w2"]
BNAMES = ["b_qkv", "b_so", "b_cq", "b_ck", "b_cv", "b_co", "b1", "b2", "b_ada"]


def _wcols(w):
    """[din, dout] dram AP -> [p, ko, dout] (feature-chunked lhsT view)."""
    return w.rearrange("(ko p) f -> p ko f", p=P)


def build_nc(upto='full'):
    nc = bacc.Bacc("TRN2", target_bir_lowering=False, debug=False)

    d = {}
    d['x'] = nc.dram_tensor("x", [N, D], F32, kind="ExternalInput").ap()
    d['c'] = nc.dram_tensor("c", [D], F32, kind="ExternalInput").ap()
    d['context'] = nc.dram_tensor("context", [S, CD], F32, kind="ExternalInput").ap()
    for nm, sh in [("w_qkv", [D, 3 * D]), ("w_so", [D, D]), ("w_cq", [D, D]),
                   ("w_ck", [CD, D]), ("w_cv", [CD, D]), ("w_co", [D, D]),
                   ("w_ada", [D, 6 * D])]:
        d[nm] = nc.dram_tensor(nm, sh, F8, kind="ExternalInput").ap()
    for nm, sh in [("w1", [D, MLP]), ("w2", [MLP, D])]:
        d[nm] = nc.dram_tensor(nm, sh, BF16, kind="ExternalInput").ap()
    for nm, sh in [("b_qkv", [3 * D]), ("b_so", [D]), ("b_cq", [D]), ("b_ck", [D]),
                   ("b_cv", [D]), ("b_co", [D]), ("b1", [MLP]), ("b2", [D]),
                   ("b_ada", [6 * D])]:
        d[nm] = nc.dram_tensor(nm, sh, F32, kind="ExternalInput").ap()
    out = nc.dram_tensor("out_x", [N, D], F32, kind="ExternalOutput").ap()
    srows = nc.dram_tensor("srows", [32, N], BF16).ap()

    with tile.TileContext(nc) as tc:
        _emit(nc, tc, d, out, srows, upto)
    nc.compile()
    return nc


def _emit(nc, tc, d, out, srows, upto='full'):
    gl = contextlib.ExitStack()          # global pools, whole-kernel lifetime
    with gl:
        const = gl.enter_context(tc.tile_pool(name="const", bufs=1))
        resid = gl.enter_context(tc.tile_pool(name="resid", bufs=2))
        wpool = gl.enter_context(tc.tile_pool(name="wpool", bufs=2))
        bigbf = gl.enter_context(tc.tile_pool(name="bigbf", bufs=3))

        # ---------- constants ----------
        ident = const.tile([P, P], F32, tag="ident")
        make_identity(nc, ident)
        identb = const.tile([P, P], BF16, tag="identb")
        nc.vector.tensor_copy(identb[:], ident[:])
        onesD_mat = const.tile([P, P], BF16, tag="onesD_mat")
        nc.vector.memset(onesD_mat[:], 1.0 / D)
        onesD_row = const.tile([P, 1], BF16, tag="onesD_row")
        nc.vector.memset(onesD_row[:], 1.0 / D)
        ones1_f = const.tile([1, P], F32, tag="ones1_f")
        nc.vector.memset(ones1_f[:], 1.0)
        ones1_b = const.tile([1, P], BF16, tag="ones1_b")
        nc.vector.memset(ones1_b[:], 1.0)
        eps_t = const.tile([P, 1], F32, tag="eps")
        nc.vector.memset(eps_t[:], EPS)
        ln4_t = const.tile([P, 1], F32, tag="ln4")
        nc.vector.memset(ln4_t[:], LN4)

        ctx8 = const.tile([P, CKD, S], F8, tag="ctx8")
        ada = const.tile([P, 48], F32, tag="ada")
        # splus8[:, g*8+k] = SA*(1+scale_g) for g=0,1 ; splus[:, k] = (1+scale_2)
        splus8 = const.tile([P, 16], F32, tag="splus8")
        sh8 = const.tile([P, 16], F32, tag="sh8")
        splus3 = const.tile([P, 8], F32, tag="splus3")
        xT = resid.tile([P, KD, N], BF16, tag="resid")

        def partial_out(t, fp32=False):
            for k in range(KD):
                nc.sync.dma_start(out[k * P:(k + 1) * P, :], t[:, k, :])

        # ---------- staging scope ----------
        st = contextlib.ExitStack()
        stg = st.enter_context(tc.tile_pool(name="stg", bufs=4))
        ps_t = st.enter_context(tc.tile_pool(name="ps_t", bufs=3, space="PSUM"))

        # silu(c)*SA in fp8, feature-major [P, KD] -- emitted first so the
        # ada GEMV can start as soon as its (large) weight DMA lands.
        cst = stg.tile([KD, P], F32, tag="cstage")
        nc.sync.dma_start(cst[:], d['c'].rearrange("(a p) -> a p", p=P))
        csil = stg.tile([KD, P], F32, tag="cstage")
        nc.scalar.activation(csil[:], cst[:], AF.Silu)
        pcs = ps_t.tile([P, 512], F32, tag="pst")
        nc.tensor.transpose(pcs[:, 0:KD], csil[:], ident[0:KD, 0:KD])
        silu_c8 = const.tile([P, KD], F8, tag="silu_c8")
        nc.vector.tensor_scalar_mul(silu_c8[:], pcs[:, 0:KD], SA)

        def bias_T(name, brow, width):
            stage = stg.tile([width, P], F32, tag="btmp")
            nc.sync.dma_start(stage[:], brow.rearrange("(a p) -> a p", p=P))
            ps = ps_t.tile([P, 512], F32, tag="pst")
            nc.tensor.transpose(ps[:, 0:width], stage[:], ident[0:width, 0:width])
            t = const.tile([P, width], F32, tag=f"bT_{name}")
            nc.vector.tensor_copy(t[:], ps[:, 0:width])
            return t

        b_qkvT = bias_T("qkv", d['b_qkv'], 24)
        b_soT = bias_T("so", d['b_so'], KD)
        b_cqT = bias_T("cq", d['b_cq'], KD)
        b_ckT = bias_T("ck", d['b_ck'], KD)
        b_coT = bias_T("co", d['b_co'], KD)
        b1T = bias_T("b1", d['b1'], MT)
        b_adaT = bias_T("ada", d['b_ada'], 48)
        b2T = bias_T("b2", d['b2'], KD)

        # ada = silu(c) @ w_ada + b_ada  -> feature-major [P, 48]
        # fp8 DoubleRow GEMV: 4 k-pair matmuls per output column.
        wada = _wcols(d['w_ada'])
        dmaengs = [nc.sync, nc.scalar, nc.gpsimd]
        for blk in range(6):
            wb = wpool.tile([P, KD, 1024], F8, tag="wblk8")
            dmaengs[blk % 3].dma_start(wb[:], wada[:, :, blk * 1024:(blk + 1) * 1024])
            for t8 in range(8):
                t = blk * 8 + t8
                ps = ps_t.tile([P, 512], F32, tag="pst")
                for k in range(KD):
                    nc.tensor.matmul(ps[:, 0:1],
                                     wb[:, k, t8 * 128:(t8 + 1) * 128],
                                     silu_c8[:, k:k + 1],
                                     start=(k == 0), stop=(k == KD - 1))
                nc.scalar.activation(ada[:, t:t + 1], ps[:, 0:1], AF.Identity,
                                     bias=b_adaT[:, t:t + 1], scale=EV)

        # x -> xT (feature-major, bf16, via bf16 PE transpose)
        for i in range(NT):
            xs = stg.tile([P, D], F32, tag="xstage")
            nc.sync.dma_start(xs[:], d['x'][i * P:(i + 1) * P, :])
            xsb = stg.tile([P, D], BF16, tag="xstageb")
            nc.scalar.activation(xsb[:], xs[:], AF.Identity)
            for jg in range(2):
                ps = ps_t.tile([P, 512], BF16, tag="pstb")
                for j4 in range(4):
                    j = jg * 4 + j4
                    nc.tensor.transpose(ps[:, j4 * 128:(j4 + 1) * 128],
                                        xsb[:, j * 128:(j + 1) * 128], identb[:])
                nc.vector.tensor_copy(
                    xT[:, jg * 4:(jg + 1) * 4, i * P:(i + 1) * P],
                    ps.rearrange("p (a b) -> p a b", a=4))

        # context -> ctx8 (fp8 x SA)
        for i in range(ST):
            cs = stg.tile([P, D], F32, tag="xstage")
            nc.sync.dma_start(cs[:, 0:CD], d['context'][i * P:(i + 1) * P, :])
            csb = stg.tile([P, D], BF16, tag="xstageb")
            nc.scalar.activation(csb[:, 0:CD], cs[:, 0:CD], AF.Identity)
            ps = ps_t.tile([P, 512], BF16, tag="pstb")
            for j in range(4):
                nc.tensor.transpose(ps[:, j * 128:(j + 1) * 128],
                                    csb[:, j * 128:(j + 1) * 128], identb[:])
            nc.vector.tensor_scalar_mul(
                ctx8[:, :, i * P:(i + 1) * P],
                ps.rearrange("p (a b) -> p a b", a=4), SA)

        # modulation scalars, with the fp8 activation scale folded in for g=0,1
        for g in range(2):
            # splus8 = SA*(ada_scale+1) ; sh8 = SA*ada_shift
            nc.vector.tensor_scalar(splus8[:, g * 8:(g + 1) * 8],
                                    ada[:, g * 16 + 8:g * 16 + 16], SA, SA,
                                    OP.mult, OP.add)
            nc.vector.tensor_scalar_mul(sh8[:, g * 8:(g + 1) * 8],
                                        ada[:, g * 16:g * 16 + 8], SA)
        nc.vector.tensor_scalar_add(splus3[:], ada[:, 40:48], 1.0)
        st.close()

        if upto == 'stage':
            partial_out(xT)
            return

        # ---------- LayerNorm + modulate ----------
        def ln_mod(x_in, g, out_dtype, splus_ap, shift_ap):
            """x_in: [P, KD, N] bf16. Returns h = LN(x)*(1+scale)+shift,
            scaled by SA when out_dtype is fp8 (folded into splus/shift)."""
            h_out = bigbf.tile([P, KD, N], out_dtype, tag="big")
            ls = contextlib.ExitStack()
            with ls:
                lnb = ls.enter_context(tc.tile_pool(name="lnb", bufs=3))
                lrows = ls.enter_context(tc.tile_pool(name="lrows", bufs=3))
                ps_mu = ls.enter_context(tc.tile_pool(name="ps_mu", bufs=1, space="PSUM"))
                ps_e2 = ls.enter_context(tc.tile_pool(name="ps_e2", bufs=1, space="PSUM"))
                # mean lands ALREADY BROADCAST across partitions: lhsT is the
                # all-ones(1/D) matrix, so every psum partition gets mean(x).
                mu_ps = ps_mu.tile([P, N], F32, tag="mups")
                e2_ps = ps_e2.tile([1, N], F32, tag="e2ps")
                for k in range(KD):
                    sq = lnb.tile([P, N], BF16, tag="lnsq")
                    nc.vector.tensor_mul(sq[:], x_in[:, k], x_in[:, k])
                    for half in range(2):
                        hs = slice(half * 512, (half + 1) * 512)
                        nc.tensor.matmul(mu_ps[:, hs], onesD_mat[:], x_in[:, k, hs],
                                         start=(k == 0), stop=(k == KD - 1))
                        nc.tensor.matmul(e2_ps[:, hs], onesD_row[:], sq[:, hs],
                                         start=(k == 0), stop=(k == KD - 1))
                murow = lrows.tile([1, N], F32, tag="row")
                nc.vector.tensor_copy(murow[:], mu_ps[0:1, :])
                var = lrows.tile([1, N], F32, tag="row")
                nc.vector.tensor_mul(var[:], murow[:], murow[:])
                nc.vector.tensor_sub(var[:], e2_ps[:], var[:])
                nc.scalar.activation(var[:], var[:], AF.Sqrt, bias=eps_t[0:1])
                rsb = lrows.tile([1, N], BF16, tag="rowb")
                with nc.allow_low_precision(reason="rstd row in bf16 is fine"):
                    nc.vector.reciprocal(rsb[:], var[:])
                # broadcast rstd across partitions with a K=1 ones matmul,
                # then keep bf16 SBUF copies so chunk ops get DVE 2x/4x.
                rs_ps = ps_mu.tile([P, N], F32, tag="rsps")
                for half in range(2):
                    hs = slice(half * 512, (half + 1) * 512)
                    nc.tensor.matmul(rs_ps[:, hs], ones1_b[:], rsb[:, hs],
                                     start=True, stop=True)
                mu_s = lnb.tile([P, N], BF16, tag="lnmub")
                nc.scalar.activation(mu_s[:], mu_ps[:], AF.Identity)
                rs_s = lnb.tile([P, N], BF16, tag="lnrsb")
                nc.scalar.activation(rs_s[:], rs_ps[:], AF.Identity)
                for k in range(KD):
                    t1 = lnb.tile([P, N], BF16, tag="lnt1")
                    nc.vector.tensor_sub(t1[:], x_in[:, k], mu_s[:])
                    nc.vector.tensor_mul(t1[:], t1[:], rs_s[:])
                    eng = nc.gpsimd if k % 2 == 0 else nc.vector
                    eng.tensor_scalar(h_out[:, k], t1[:],
                                      splus_ap[:, k:k + 1],
                                      shift_ap[:, k:k + 1],
                                      OP.mult, OP.add)
            return h_out

        # ---------- fp8 DoubleRow transposed projection ----------
        def proj8(ps_mm, w_cols, kdin, act8, n_free, dout, evict):
            nq = max(1, n_free // 256)
            for blk in range(dout // 1024):
                wb = wpool.tile([P, kdin, 1024], F8, tag="wblk8")
                nc.sync.dma_start(wb[:], w_cols[:, :, blk * 1024:(blk + 1) * 1024])
                for t8 in range(8):
                    ps = ps_mm.tile([P, N], F32, tag="pmm")
                    for qq in range(nq):
                        qs = slice(qq * 256, qq * 256 + min(256, n_free))
                        for kp in range(kdin // 2):
                            nc.tensor.matmul(
                                ps[:, qs],
                                wb[:, 2 * kp:2 * kp + 2, t8 * 128:(t8 + 1) * 128],
                                act8[:, 2 * kp:2 * kp + 2, qs],
                                start=(kp == 0), stop=(kp == kdin // 2 - 1),
                                perf_mode=DR)
                    evict(blk * 8 + t8, ps)

        # ---------- token-major V projection (fused ones column) ----------
        def proj_V8(ps_mm, w_cols, kdin, act8, m_tiles, v65, vbias8):
            wb = wpool.tile([P, kdin, 1024], F8, tag="wblk8")
            nc.sync.dma_start(wb[:], w_cols[:])
            for dblk in range(4):
                for i in range(m_tiles):
                    ps = ps_mm.tile([P, N], F32, tag="pmm")
                    for kp in range(kdin // 2):
                        nc.tensor.matmul(
                            ps[:, 0:256],
                            act8[:, 2 * kp:2 * kp + 2, i * 128:(i + 1) * 128],
                            wb[:, 2 * kp:2 * kp + 2, dblk * 256:(dblk + 1) * 256],
                            start=(kp == 0), stop=(kp == kdin // 2 - 1),
                            perf_mode=DR)
                    # v65 = SV*(ps/(SW*SA) + b) = ps*(SV*EV) + SV*b
                    nc.vector.scalar_tensor_tensor(
                        v65[:, i, dblk * 4:(dblk + 1) * 4, 0:64],
                        ps[:, 0:256].rearrange("p (h e) -> p h e", h=4),
                        SV * EV,
                        vbias8[:, dblk * 256:(dblk + 1) * 256]
                        .rearrange("p (h e) -> p h e", h=4),
                        OP.mult, OP.add)
            nc.vector.memset(v65[:, :, :, 64:65], 1.0)

        # ---------- attention core (fp8 DoubleRow PV) ----------
        def attention(q_T, kv_T, v65, m_tiles, o8, srow_base):
            """q_T/kv_T: bf16 true-scale. v65: fp8 (v*SV | ones). o8: fp8 out
            = SA * softmax(qk/8) v. PV runs DoubleRow over k-tile pairs."""
            npair = m_tiles // 2
            at = contextlib.ExitStack()
            with at:
                expp = at.enter_context(tc.tile_pool(name="expp", bufs=6))
                arows = at.enter_context(tc.tile_pool(name="arows", bufs=2))
                ps_lg = at.enter_context(tc.tile_pool(name="ps_lg", bufs=2, space="PSUM"))
                ps_pv = at.enter_context(tc.tile_pool(name="ps_pv", bufs=2, space="PSUM"))
                for h in range(H):
                    pr, off = h // 2, (h % 2) * 64
                    pv = ps_pv.tile([P, N], F32, tag="pv")
                    # software pipeline over (pair, half): logits+exp run one
                    # pair ahead of the PV accumulation.
                    items = npair * 2
                    exs = [None] * items

                    def lgexp(i):
                        mp, half = i // 2, i % 2
                        hs = slice(half * 512, (half + 1) * 512)
                        ex = expp.tile([P, 2, 512], F8, tag="expT", name=f"ex{h}_{i}")
                        lg = ps_lg.tile([P, 2, 512], F32, tag="lg",
                                        name=f"lg{h}_{i}")
                        for m2 in range(2):
                            mt = 2 * mp + m2
                            nc.tensor.matmul(
                                lg[:, m2, :],
                                kv_T[off:off + 64, pr, mt * 128:(mt + 1) * 128],
                                q_T[off:off + 64, pr, hs],
                                start=True, stop=True)
                        # SP * exp(logits/8) = exp(logits/8 + ln SP)
                        nc.scalar.activation(ex[:], lg[:], AF.Exp,
                                             bias=ln4_t[:], scale=ASCALE)
                        exs[i] = ex

                    def pvacc(i):
                        mp, half = i // 2, i % 2
                        for qq in range(2):
                            qs = slice(half * 512 + qq * 256,
                                       half * 512 + (qq + 1) * 256)
                            nc.tensor.matmul(
                                pv[0:65, qs],
                                v65[:, 2 * mp:2 * mp + 2, h, :],
                                exs[i][:, :, qq * 256:(qq + 1) * 256],
                                start=(mp == 0), stop=(mp == npair - 1),
                                perf_mode=DR)

                    for i in range(items + 2):
                        if i < items:
                            lgexp(i)
                        if i >= 2:
                            pvacc(i - 2)
                    # o8 = pv_rows * (1/pv[64]) ; scales: pv_rows = SP*SV*o*den,
                    # pv[64] = SP*den  ->  o8 = SV*o = SA*o (SV == SA).
                    rec = arows.tile([1, N], BF16, tag="rowb")
                    with nc.allow_low_precision(reason="softmax denom in bf16"):
                        nc.vector.reciprocal(rec[:], pv[64:65, :])
                    r = srow_base + h
                    nc.sync.dma_start(srows[r:r + 1, :], rec[:])
                    bcs = arows.tile([64, N], BF16, tag="bcs")
                    nc.sync.dma_start(bcs[:], srows[r:r + 1, :].partition_broadcast(64))
                    nc.vector.tensor_mul(o8[off:off + 64, pr, :], pv[0:64, :], bcs[:])

        # ================= self-attention =================
        h1 = ln_mod(xT, 0, F8, splus8[:, 0:8], sh8[:, 0:8])

        qT = bigbf.tile([P, KD, N], BF16, tag="big")
        kT = bigbf.tile([P, KD, N], BF16, tag="big")

        sa_es = contextlib.ExitStack()
        vp = sa_es.enter_context(tc.tile_pool(name="vp", bufs=1))
        vb = sa_es.enter_context(tc.tile_pool(name="vb", bufs=1))
        v65 = vp.tile([P, NT, H, 65], F8, tag="v65")
        vbias8 = vb.tile([P, D], F32, tag="vbias")
        nc.sync.dma_start(vbias8[:],
                          d['b_qkv'][2 * D:3 * D][None, :].partition_broadcast(P))
        nc.vector.tensor_scalar_mul(vbias8[:], vbias8[:], SV)

        qkv_ps = contextlib.ExitStack()
        ps_mm = qkv_ps.enter_context(tc.tile_pool(name="ps_mm", bufs=2, space="PSUM"))

        def ev_qk(t, ps):
            dst = qT if t < 8 else kT
            nc.vector.tensor_scalar(dst[:, t % 8, :], ps[:], EV,
                                    b_qkvT[:, t:t + 1], OP.mult, OP.add)
        proj_V8(ps_mm, _wcols(d['w_qkv'])[:, :, 2 * D:3 * D], KD, h1, NT, v65, vbias8)
        proj8(ps_mm, _wcols(d['w_qkv'])[:, :, 0:2 * D], KD, h1, N, 2 * D, ev_qk)
        qkv_ps.close()

        if upto == 'qkv':
            partial_out(xT)
            return
        o8 = bigbf.tile([P, KD, N], F8, tag="big")
        attention(qT, kT, v65, NT, o8, 0)
        sa_es.close()
        if upto == 'sa':
            partial_out(xT)
            return

        x2T = resid.tile([P, KD, N], BF16, tag="resid")
        so_ps = contextlib.ExitStack()
        ps_mm = so_ps.enter_context(tc.tile_pool(name="ps_mm", bufs=2, space="PSUM"))
        sstage = so_ps.enter_context(tc.tile_pool(name="sstage", bufs=3))

        def ev_so(t, ps):
            tmp = sstage.tile([P, N], BF16, tag="sot")
            nc.scalar.activation(tmp[:], ps[:], AF.Identity,
                                 bias=b_soT[:, t:t + 1], scale=EV)
            nc.vector.tensor_add(x2T[:, t, :], tmp[:], xT[:, t, :])
        proj8(ps_mm, _wcols(d['w_so']), KD, o8, N, D, ev_so)
        so_ps.close()

        # ================= cross-attention =================
        # ck/cv depend only on ctx8 -> emit BEFORE LN2 so the PE has work
        # during the LN serial chain.
        ca_es = contextlib.ExitStack()
        kp_ = ca_es.enter_context(tc.tile_pool(name="kp", bufs=1))
        vp2 = ca_es.enter_context(tc.tile_pool(name="vp2", bufs=1))
        vb2 = ca_es.enter_context(tc.tile_pool(name="vb2", bufs=1))
        ckT = kp_.tile([P, KD, S], BF16, tag="ckT")
        cv65 = vp2.tile([P, ST, H, 65], F8, tag="cv65")
        cvbias8 = vb2.tile([P, D], F32, tag="cvbias")
        nc.sync.dma_start(cvbias8[:], d['b_cv'][None, :].partition_broadcast(P))
        nc.vector.tensor_scalar_mul(cvbias8[:], cvbias8[:], SV)

        ckcv_ps = contextlib.ExitStack()
        ps_kv = ckcv_ps.enter_context(tc.tile_pool(name="ps_kv", bufs=1, space="PSUM"))

        def ev_ck(t, ps):
            nc.scalar.activation(ckT[:, t, :], ps[:, 0:S], AF.Identity,
                                 bias=b_ckT[:, t:t + 1], scale=EV)
        proj8(ps_kv, _wcols(d['w_ck']), CKD, ctx8, S, D, ev_ck)
        proj_V8(ps_kv, _wcols(d['w_cv']), CKD, ctx8, ST, cv65, cvbias8)

        h2 = ln_mod(x2T, 1, F8, splus8[:, 8:16], sh8[:, 8:16])
        ckcv_ps.close()

        cqT = bigbf.tile([P, KD, N], BF16, tag="big")
        ca_ps = contextlib.ExitStack()
        ps_mm = ca_ps.enter_context(tc.tile_pool(name="ps_mm", bufs=2, space="PSUM"))

        def ev_cq(t, ps):
            nc.scalar.activation(cqT[:, t, :], ps[:], AF.Identity,
                                 bias=b_cqT[:, t:t + 1], scale=EV)
        proj8(ps_mm, _wcols(d['w_cq']), KD, h2, N, D, ev_cq)
        ca_ps.close()

        w1blk0 = wpool.tile([P, KD, 1024], BF16, tag="wblkb")
        nc.sync.dma_start(w1blk0[:], _wcols(d['w1'])[:, :, 0:1024])

        co8 = bigbf.tile([P, KD, N], F8, tag="big")
        attention(cqT, ckT, cv65, ST, co8, 16)
        ca_es.close()

        x3T = resid.tile([P, KD, N], BF16, tag="resid")
        co_ps = contextlib.ExitStack()
        ps_mm = co_ps.enter_context(tc.tile_pool(name="ps_mm", bufs=2, space="PSUM"))
        cstage = co_ps.enter_context(tc.tile_pool(name="cstage", bufs=3))

        def ev_co(t, ps):
            tmp = cstage.tile([P, N], BF16, tag="cot")
            nc.scalar.activation(tmp[:], ps[:], AF.Identity,
                                 bias=b_coT[:, t:t + 1], scale=EV)
            nc.vector.tensor_add(x3T[:, t, :], tmp[:], x2T[:, t, :])
        proj8(ps_mm, _wcols(d['w_co']), KD, co8, N, D, ev_co)
        co_ps.close()

        if upto == 'ca':
            partial_out(x3T)
            return
        # ================= FFN (bf16, g SBUF-resident) =================
        # token-major copy of x3, emitted HERE so the PE transposes fill the
        # LN3 serial window. b2 is accumulated into the w2 psum later.
        xtok = [bigbf.tile([P, 4, D], BF16, tag="big", name=f"xtok{_i}")
                for _i in range(2)]
        tt_es = contextlib.ExitStack()
        ps_tt = tt_es.enter_context(tc.tile_pool(name="ps_tt", bufs=2, space="PSUM"))
        for i in range(NT):
            dst = xtok[i // 4]
            for jg in range(2):
                ps = ps_tt.tile([P, 512], BF16, tag="ptt")
                for j4 in range(4):
                    j = jg * 4 + j4
                    nc.tensor.transpose(ps[:, j4 * 128:(j4 + 1) * 128],
                                        x3T[:, j, i * P:(i + 1) * P], identb[:])
                nc.scalar.activation(dst[:, i % 4, jg * 512:(jg + 1) * 512], ps[:],
                                     AF.Identity)
        tt_es.close()
        h3 = ln_mod(x3T, 2, BF16, splus3, ada[:, 32:40])

        ffn_es = contextlib.ExitStack()
        gp = ffn_es.enter_context(tc.tile_pool(name="gp", bufs=1))
        g = gp.tile([P, MT, N], BF16, tag="g")

        w1_es = contextlib.ExitStack()
        ps_mm = w1_es.enter_context(tc.tile_pool(name="ps_mm", bufs=2, space="PSUM"))
        for blk in range(4):
            if blk == 0:
                wb = w1blk0
            else:
                wb = wpool.tile([P, KD, 1024], BF16, tag="wblkb")
                nc.sync.dma_start(wb[:], _wcols(d['w1'])[:, :, blk * 1024:(blk + 1) * 1024])
            for t8 in range(8):
                t = blk * 8 + t8
                ps = ps_mm.tile([P, N], F32, tag="pmm")
                for half in range(2):
                    hs = slice(half * 512, (half + 1) * 512)
                    for k in range(KD):
                        nc.tensor.matmul(ps[:, hs],
                                         wb[:, k, t8 * 128:(t8 + 1) * 128],
                                         h3[:, k, hs],
                                         start=(k == 0), stop=(k == KD - 1))
                nc.scalar.activation(g[:, t, :], ps[:], AF.Gelu, bias=b1T[:, t:t + 1])
        w1_es.close()

        if upto == 'w1':
            partial_out(x3T)
            return
        w2_es = contextlib.ExitStack()
        outst = w2_es.enter_context(tc.tile_pool(name="outst", bufs=3))
        ps_po = w2_es.enter_context(tc.tile_pool(name="ps_po", bufs=4, space="PSUM"))

        b2row_f = const.tile([1, D], F32, tag="b2rowf")
        nc.sync.dma_start(b2row_f[:], d['b2'][None, :])
        b2row = const.tile([1, D], BF16, tag="b2row")
        nc.vector.tensor_copy(b2row[:], b2row_f[:])
        onescol = const.tile([1, P], BF16, tag="onescol")
        nc.vector.memset(onescol[:], 1.0)

        w2cols = d['w2'].rearrange("(ko p) f -> p ko f", p=P)
        for dq in range(4):
            w2q = wpool.tile([P, MT, 256], BF16, tag="wblkb")
            nc.sync.dma_start(w2q[:], w2cols[:, :, dq * 256:(dq + 1) * 256])
            for i in range(NT):
                po = ps_po.tile([P, 256], F32, tag="po")
                nc.tensor.matmul(po[:], onescol[:],
                                 b2row[:, dq * 256:(dq + 1) * 256],
                                 start=True, stop=False)
                for k in range(MT):
                    nc.tensor.matmul(po[:], g[:, k, i * 128:(i + 1) * 128],
                                     w2q[:, k, :],
                                     start=False, stop=(k == MT - 1))
                ost = outst.tile([P, 256], F32, tag="ost")
                nc.vector.tensor_add(
                    ost[:], po[:],
                    xtok[i // 4][:, i % 4, dq * 256:(dq + 1) * 256])
                nc.sync.dma_start(out[i * P:(i + 1) * P, dq * 256:(dq + 1) * 256],
                                  ost[:])
        w2_es.close()
        ffn_es.close()


_NC = None


def _get_nc():
    global _NC
    if _NC is None:
        _NC = build_nc()
    return _NC


def make_in_maps(inputs):
    shared = {}
    for nm in F8_WNAMES:
        shared[nm] = np.ascontiguousarray(
            (np.asarray(inputs[nm], dtype=np.float32) * SW)
            .astype(ml_dtypes.float8_e4m3))
    for nm in BF_WNAMES:
        shared[nm] = np.ascontiguousarray(
            np.asarray(inputs[nm]).astype(ml_dtypes.bfloat16))
    for nm in BNAMES:
        shared[nm] = np.ascontiguousarray(np.asarray(inputs[nm], dtype=np.float32))
    x = np.asarray(inputs['x'], dtype=np.float32)
    c = np.asarray(inputs['c'], dtype=np.float32)
    ctxt = np.asarray(inputs['context'], dtype=np.float32)
    in_maps = []
    for i in range(NCORES):
        m = dict(shared)
        m['x'] = np.ascontiguousarray(x[i])
        m['c'] = np.ascontiguousarray(c[i])
        m['context'] = np.ascontiguousarray(ctxt[i])
        in_maps.append(m)
    return in_maps


def kernel(**inputs):
    nc = _get_nc()
    in_maps = make_in_maps(inputs)
    res = run_bass_kernel_spmd(nc, in_maps, core_ids=list(range(NCORES)))
    return np.stack([res.results[i]["out_x"] for i in range(NCORES)]).astype(np.float32)


if __name__ == "__main__":
    data = np.load("/root/problem/inputs.npz")
    out = kernel(**{k: data[k] for k in data.files})
    gold = np.load("/root/problem/gold64.npy")
    err = np.abs(out - gold)
    print("max abs err:", err.max(), " rel:", err.max() / np.abs(gold).max())
